# revision 7
# baseline (speedup 1.0000x reference)
"""Causal self-attention (B=4, T=2048, D=1024, H=16) on 8 Trainium2 NeuronCores.

Sharding: data-parallel over batch (4) x tensor-parallel over heads (2 groups
of 8 heads) = 8 cores. Each core computes q/k/v projections for its 8 heads,
head-local attention, and a partial out-projection; the host sums the two
partials per batch element (the out_proj all-reduce).

Numerics/performance scheme:
  - QKV projections run as fp8e4m3 DoubleRow matmuls with an error-compensated
    3-term expansion: x = x8 + xr/16, W = W8 + Wr/16 (residuals pre-scaled by
    16 on the host to dodge fp8 subnormal underflow);
    q = W8.x8 + (W8.xr + Wr.x8)/16.  The two PSUM groups (unit + 1/16 scale)
    are combined by a DVE scalar_tensor_tensor; ScalarE quantizes q+bias to
    fp8 (hi) and GpSimd writes the fp8 residual (lo).
  - S = K^T Q runs as ONE fp8 DoubleRow matmul per (head, k-tile) computing
    the exact 4-product expansion (k8+kr)(q8+qr): the moving operand is a
    [128, 2, N] "quad" with q-hi/q-lo on partition halves duplicated across
    the two DoubleRow slots, the stationary is [k-hi; k-lo] stacked on
    partition halves with a stride-0 slot dim.  Cost N/2 instead of 2N.
  - Attention tail (exp, P@V, normalize, out-proj) is fp32r as before.
"""
import numpy as np
import ml_dtypes

import concourse.bass as bass
import concourse.bacc as bacc
import concourse.tile as tile
from concourse import mybir
from concourse.bass_utils import run_bass_kernel_spmd

F32 = mybir.dt.float32
F32R = mybir.dt.float32r
FP8 = mybir.dt.float8e4
E4NP = ml_dtypes.float8_e4m3
EXP = mybir.ActivationFunctionType.Exp
IDN = mybir.ActivationFunctionType.Identity
MULT = mybir.AluOpType.mult
ADD = mybir.AluOpType.add
SUB = mybir.AluOpType.subtract
DR = mybir.MatmulPerfMode.DoubleRow

B, T, D = 4, 2048, 1024
H = 16            # total heads
HD = 64           # head dim
HC = 8            # heads per core
NP = 4            # head pairs per core
NEG = -1.0e6      # additive mask value (exp -> 0 after *0.125)
RS = 16.0         # residual pre-scale

_CACHE = {}


def _build():
    nc = bacc.Bacc("TRN2", target_bir_lowering=False, debug=False,
                   enable_asserts=False)
    dt_in = {}

    def din(name, shape, dt=F32):
        dt_in[name] = nc.dram_tensor(name, shape, dt, kind="ExternalInput").ap()
        return dt_in[name]

    x8d = din("x8", [D, T], FP8)          # fp8(x[b].T)
    xrd = din("xr", [D, T], FP8)          # fp8((x.T - x8)*16)
    wq8 = din("wq8", [128, 4, 2, 512], FP8)
    wqr = din("wqr", [128, 4, 2, 512], FP8)
    wk8 = din("wk8", [128, 4, 2, 512], FP8)
    wkr = din("wkr", [128, 4, 2, 512], FP8)
    wv8 = din("wv8", [128, 4, 2, 512], FP8)
    wvr = din("wvr", [128, 4, 2, 512], FP8)
    bvm = din("bvm", [1, 2, 512], FP8)    # [bv*16 ; zeros]
    wo = din("wo", [512, D])              # Wo[:, cols].T
    bqt = din("bqt", [128, NP])
    bkt = din("bkt", [128, NP])
    bo = din("bo", [D])
    tri = din("tri", [128, 128])          # causal 0/1 multiplicative band mask
    keyb = din("keyb", [128, 16])         # key-padding additive bias per k-tile
    qmt = din("qmt", [128, 16])           # query mask, token-major
    out = nc.dram_tensor("out", [T, D], F32, kind="ExternalOutput").ap()

    def bcast128(ap):
        return bass.AP(tensor=ap.tensor, offset=ap.offset, ap=[[0, 128]] + ap.ap)

    def slot0(ap):
        # insert a stride-0 DoubleRow slot dim after the partition dim
        return bass.AP(tensor=ap.tensor, offset=ap.offset,
                       ap=[ap.ap[0], [0, 2]] + ap.ap[1:])

    with tile.TileContext(nc) as tc:
        cpool = tc.alloc_tile_pool(name="const", bufs=1)
        tri_t = cpool.tile([128, 128], F32)
        keyb_t = cpool.tile([128, 16], F32)
        qmt_t = cpool.tile([128, 16], F32)
        bqt_t = cpool.tile([128, NP], F32)
        bkt_t = cpool.tile([128, NP], F32)
        borep = cpool.tile([128, D], F32)
        bvm_t = cpool.tile([1, 2, 512], FP8)
        ones_st = cpool.tile([1, 2, 128], FP8)
        vone_f = cpool.tile([128, HC, 16], F32)
        nc.sync.dma_start(out=tri_t, in_=tri)
        nc.sync.dma_start(out=keyb_t, in_=keyb)
        nc.sync.dma_start(out=qmt_t, in_=qmt)
        nc.sync.dma_start(out=bqt_t, in_=bqt)
        nc.sync.dma_start(out=bkt_t, in_=bkt)
        nc.sync.dma_start(out=bvm_t, in_=bvm)
        nc.vector.memset(ones_st[:, 0, :], 1.0)
        nc.vector.memset(ones_st[:, 1, :], 0.0)
        nc.vector.memset(vone_f, 1.0)

        wopool = tc.alloc_tile_pool(name="wop", bufs=1)
        wo_t = wopool.tile([128, 4, D], F32R)

        # persistent fp8 attention operands
        qkpool = tc.alloc_tile_pool(name="qk8", bufs=1)
        q_quad = qkpool.tile([128, HC, 2, T], FP8)   # per head: [q8;qr] x slot-dup
        k_stk = qkpool.tile([128, HC, T], FP8)       # per head: [k8;kr] stacked
        qkvpool = tc.alloc_tile_pool(name="qkv", bufs=1)
        vA = qkvpool.tile([128, HC, 16, HD + 1], F32R, tag="vA")
        nc.vector.tensor_copy(vA[:, :, :, HD], vone_f)

        # ---------------- Phase 1: QKV projections (fp8 3-set DR) -----------
        with (
            tc.tile_pool(name="wts", bufs=1) as wpool,
            tc.tile_pool(name="xts", bufs=2) as xpool,
            tc.tile_pool(name="prs", bufs=1) as prpool,
            tc.tile_pool(name="tmp", bufs=3) as tpool,
            tc.tile_pool(name="psA", bufs=2, space="PSUM") as psA,
            tc.tile_pool(name="psB", bufs=2, space="PSUM") as psB,
            tc.tile_pool(name="psvA", bufs=2, space="PSUM") as psvA,
            tc.tile_pool(name="psvB", bufs=2, space="PSUM") as psvB,
        ):
            q8p = prpool.tile([128, NP, T], FP8, tag="q8p")
            qrp = prpool.tile([128, NP, T], FP8, tag="qrp")
            k8p = prpool.tile([128, NP, T], FP8, tag="k8p")
            krp = prpool.tile([128, NP, T], FP8, tag="krp")

            wq8_t = wpool.tile([128, 4, 2, 512], FP8, tag="w0")
            wqr_t = wpool.tile([128, 4, 2, 512], FP8, tag="w1")
            wk8_t = wpool.tile([128, 4, 2, 512], FP8, tag="w2")
            wkr_t = wpool.tile([128, 4, 2, 512], FP8, tag="w3")
            wv8_t = wpool.tile([128, 4, 2, 512], FP8, tag="w4")
            wvr_t = wpool.tile([128, 4, 2, 512], FP8, tag="w5")

            def load_w(wdram, wt, eng):
                for kd in range(4):
                    eng.dma_start(out=wt[:, kd, :, :], in_=wdram[:, kd, :, :])

            SLICES = [(0, 256), (256, 256), (512, 256), (768, 256), (1024, 512), (1536, 512)]

            def load_x(si):
                t0, tl = SLICES[si]
                x8t = xpool.tile([128, 4, 2, 512], FP8, tag="x8", name=f"x8_{si}",
                                 padded_shape=[128, 4, 2, 512])
                xrt = xpool.tile([128, 4, 2, 512], FP8, tag="xr", name=f"xr_{si}",
                                 padded_shape=[128, 4, 2, 512])
                for kd in range(4):
                    for i in range(2):
                        r0 = 256 * kd + 128 * i
                        eng = nc.scalar if i else nc.sync
                        eng.dma_start(out=x8t[:, kd, i, 0:tl],
                                      in_=x8d[r0:r0 + 128, t0:t0 + tl])
                        eng2 = nc.sync if i else nc.scalar
                        eng2.dma_start(out=xrt[:, kd, i, 0:tl],
                                       in_=xrd[r0:r0 + 128, t0:t0 + tl])
                return x8t, xrt

            load_w(wq8, wq8_t, nc.gpsimd)
            xts_next = load_x(0)
            load_w(wk8, wk8_t, nc.gpsimd)
            load_w(wqr, wqr_t, nc.gpsimd)
            load_w(wkr, wkr_t, nc.gpsimd)
            load_w(wv8, wv8_t, nc.gpsimd)
            load_w(wvr, wvr_t, nc.gpsimd)
            nc.gpsimd.dma_start(out=borep, in_=bcast128(bo))
            nc.gpsimd.dma_start(out=wo_t, in_=wo.rearrange("(g p) n -> p g n", p=128).bitcast(F32R))

            for si in range(len(SLICES)):
                t0, tl = SLICES[si]
                x8t, xrt = xts_next
                if si + 1 < len(SLICES):
                    xts_next = load_x(si + 1)
                for (w8t, wrt, bias_t, hip, lop) in (
                        (wq8_t, wqr_t, bqt_t, q8p, qrp),
                        (wk8_t, wkr_t, bkt_t, k8p, krp)):
                    for g in range(NP):
                        gc = slice(g * 128, (g + 1) * 128)
                        pa = psA.tile([128, 512], F32, tag="pa")
                        for kd in range(4):
                            nc.tensor.matmul(pa[:, 0:tl], w8t[:, kd, :, gc],
                                             x8t[:, kd, :, 0:tl], perf_mode=DR,
                                             start=(kd == 0), stop=(kd == 3))
                        pb = psB.tile([128, 512], F32, tag="pb")
                        for kd in range(4):
                            nc.tensor.matmul(pb[:, 0:tl], w8t[:, kd, :, gc],
                                             xrt[:, kd, :, 0:tl], perf_mode=DR,
                                             start=(kd == 0), stop=False)
                        for kd in range(4):
                            nc.tensor.matmul(pb[:, 0:tl], wrt[:, kd, :, gc],
                                             x8t[:, kd, :, 0:tl], perf_mode=DR,
                                             start=False, stop=(kd == 3))
                        qtmp = tpool.tile([128, 512], F32, tag="qt", name=f"qt{si}_{g}")
                        nc.vector.scalar_tensor_tensor(qtmp[:, 0:tl], pb[:, 0:tl],
                                                       1.0 / RS, pa[:, 0:tl],
                                                       op0=MULT, op1=ADD)
                        nc.scalar.activation(hip[:, g, t0:t0 + tl], qtmp[:, 0:tl],
                                             IDN, bias=bias_t[:, g:g + 1])
                        nc.gpsimd.scalar_tensor_tensor(lop[:, g, t0:t0 + tl],
                                                       qtmp[:, 0:tl],
                                                       bias_t[:, g:g + 1],
                                                       hip[:, g, t0:t0 + tl],
                                                       op0=ADD, op1=SUB)
                for tt in range(tl // 128):
                    gtt = (t0 // 128) + tt
                    ts = slice(tt * 128, (tt + 1) * 128)
                    va_ = psvA.tile([128, 512], F32, tag="pva", name=f"pva{gtt}")
                    for kd in range(4):
                        nc.tensor.matmul(va_, x8t[:, kd, :, ts], wv8_t[:, kd, :, :],
                                         perf_mode=DR, start=(kd == 0), stop=(kd == 3))
                    vb_ = psvB.tile([128, 512], F32, tag="pvb", name=f"pvb{gtt}")
                    for kd in range(4):
                        nc.tensor.matmul(vb_, xrt[:, kd, :, ts], wv8_t[:, kd, :, :],
                                         perf_mode=DR, start=(kd == 0), stop=False)
                    for kd in range(4):
                        nc.tensor.matmul(vb_, x8t[:, kd, :, ts], wvr_t[:, kd, :, :],
                                         perf_mode=DR, start=False, stop=False)
                    nc.tensor.matmul(vb_, ones_st, bvm_t, perf_mode=DR,
                                     start=False, stop=True)
                    nc.vector.scalar_tensor_tensor(
                        vA[:, :, gtt, 0:HD],
                        vb_.rearrange("p (h d) -> p h d", h=HC), 1.0 / RS,
                        va_.rearrange("p (h d) -> p h d", h=HC),
                        op0=MULT, op1=ADD)

                # build per-head S operands as soon as each T-half is done:
                # q_quad [q8;qr] dup'd across slots, k_stk [k8;kr] stacked
                if si in (3, 5):
                    t0h, t1h = (0, 1024) if si == 3 else (1024, 2048)
                    for h in range(HC):
                        pr, hh = h // 2, h % 2
                        half = slice(64 * hh, 64 * hh + 64)
                        qeng = (nc.sync, nc.scalar, nc.gpsimd)[h % 3]
                        qeng.dma_start(out=k_stk[0:64, h, t0h:t1h], in_=k8p[half, pr, t0h:t1h])
                        qeng.dma_start(out=k_stk[64:128, h, t0h:t1h], in_=krp[half, pr, t0h:t1h])
                        qeng.dma_start(out=q_quad[0:64, h, 0, t0h:t1h], in_=q8p[half, pr, t0h:t1h])
                        qeng.dma_start(out=q_quad[0:64, h, 1, t0h:t1h], in_=qrp[half, pr, t0h:t1h])
                        qeng.dma_start(out=q_quad[64:128, h, 0, t0h:t1h], in_=q8p[half, pr, t0h:t1h])
                        qeng.dma_start(out=q_quad[64:128, h, 1, t0h:t1h], in_=qrp[half, pr, t0h:t1h])



        # ---------------- Phase 2: attention ----------------
        ypool = tc.alloc_tile_pool(name="yT", bufs=1)
        yT = ypool.tile([128, NP, T], F32R)
        def dim0bc(ap, n):
            # insert a stride-0 dim after the partition dim (broadcast)
            return bass.AP(tensor=ap.tensor, offset=ap.offset,
                           ap=[ap.ap[0], [0, n]] + ap.ap[1:])

        with (
            tc.tile_pool(name="ptile", bufs=8) as ppool,
            tc.tile_pool(name="dro", bufs=2) as dpool,
            tc.tile_pool(name="rec", bufs=2) as rpool,
            tc.tile_pool(name="pss", bufs=2, space="PSUM") as pss,
            tc.tile_pool(name="psy", bufs=2, space="PSUM") as psy,
        ):
            prev_tail = None
            for pr in range(NP):
                for qt in range(4):
                    q0 = qt * 512
                    nk = qt * 4 + 4
                    # two-bank tiles: head A in cols [0,512), head B in [512,1024)
                    ys2 = psy.tile([HD + 1, 1024], F32, tag="y", name=f"y{pr}_{qt}")
                    for kt in range(nk):
                        c = kt * 128 - q0
                        mm_lo = 0 if c < 0 else min(c, 256)
                        lo = max(c, 0)
                        s2 = pss.tile([128, 1024], F32, tag="s", name=f"s{pr}_{qt}_{kt}")
                        with tc.high_priority(offset=48):
                            for hh in range(2):
                                h = 2 * pr + hh
                                nc.tensor.matmul(
                                    s2[:, hh * 512 + mm_lo:hh * 512 + 512],
                                    slot0(k_stk[:, h, kt * 128:(kt + 1) * 128]),
                                    q_quad[:, h, :, q0 + mm_lo:q0 + 512],
                                    perf_mode=DR, start=True, stop=True)
                        s2v = s2.rearrange("p (b n) -> p b n", b=2)
                        p2 = ppool.tile([128, 1024], F32R, tag="p", name=f"p{pr}_{qt}_{kt}")
                        p2v = p2.rearrange("p (b n) -> p b n", b=2)
                        with tc.high_priority(offset=96):
                            nc.scalar.activation(p2v[:, :, lo:512],
                                                 s2v[:, :, lo:512], EXP,
                                                 bias=keyb_t[:, kt:kt + 1], scale=0.125)
                        if c >= 0:
                            nc.vector.tensor_mul(p2v[:, :, lo:lo + 128],
                                                 p2v[:, :, lo:lo + 128],
                                                 dim0bc(tri_t, 2))
                        for hh in range(2):
                            nc.tensor.matmul(ys2[:, hh * 512 + lo:hh * 512 + 512],
                                             vA[:, 2 * pr + hh, kt, :],
                                             p2[:, hh * 512 + lo:hh * 512 + 512],
                                             start=(kt == 0), stop=(kt == nk - 1),
                                             skip_group_check=True)
                    def tail(pr=pr, qt=qt, q0=q0, ys2=ys2):
                        drow = dpool.tile([1, 1024], F32, tag="dc", name=f"dc{pr}_{qt}")
                        nc.vector.tensor_copy(drow, ys2[HD:HD + 1, :])
                        rec1 = dpool.tile([1, 1024], F32, tag="d", name=f"d{pr}_{qt}")
                        nc.vector.reciprocal_approx_fast(rec1, drow)
                        rec2 = rpool.tile([HD, 1024], F32, tag="r", name=f"r{pr}_{qt}")
                        nc.gpsimd.partition_broadcast(rec2, rec1)
                        for hh in range(2):
                            nc.vector.tensor_mul(yT[hh * 64:hh * 64 + 64, pr, q0:q0 + 512],
                                                 ys2[0:HD, hh * 512:hh * 512 + 512],
                                                 rec2[:, hh * 512:hh * 512 + 512])
                    # defer this iteration's denominator/normalize tail until after
                    # the next iteration's matmul loop so its DVE work doesn't
                    # delay the next wave of exps
                    if prev_tail is not None:
                        prev_tail()
                    prev_tail = tail
            prev_tail()

        # ---------------- Phase 3: out projection ----------------
        with (
            tc.tile_pool(name="ob", bufs=4) as opool,
            tc.tile_pool(name="pso", bufs=4, space="PSUM") as psop,
        ):
            for tt in range(16):
                for oh in range(2):
                    po = psop.tile([128, 512], F32, tag="o", name=f"po{tt}_{oh}")
                    for g in range(NP):
                        nc.tensor.matmul(po, yT[:, g, tt * 128:(tt + 1) * 128],
                                         wo_t[:, g, oh * 512:(oh + 1) * 512],
                                         start=(g == 0), stop=(g == NP - 1))
                    ob = opool.tile([128, 512], F32, tag="ob", name=f"ob{tt}_{oh}")
                    nc.vector.scalar_tensor_tensor(ob, po, qmt_t[:, tt:tt + 1],
                                                   borep[:, oh * 512:(oh + 1) * 512],
                                                   op0=MULT, op1=ADD)
                    nc.sync.dma_start(out=out[tt * 128:(tt + 1) * 128, oh * 512:(oh + 1) * 512],
                                      in_=ob)
        for p in (ypool, qkvpool, qkpool, wopool, cpool):
            p.release()
    nc.compile()
    return nc


def _host_inputs(x, attention_mask, Wqkv, bqkv, Wo, bo):
    """Build the 8 per-core input maps."""
    x = np.asarray(x, dtype=np.float32)
    attention_mask = np.asarray(attention_mask)
    Wqkv = np.asarray(Wqkv, dtype=np.float32)
    bqkv = np.asarray(bqkv, dtype=np.float32)
    Wo = np.asarray(Wo, dtype=np.float32)
    bo = np.asarray(bo, dtype=np.float32)

    tri = (np.arange(128)[:, None] <= np.arange(128)[None, :]).astype(np.float32)
    zeros_bo = np.zeros_like(bo)

    def hilo(a):
        hi = a.astype(E4NP)
        lo = ((a - hi.astype(np.float32)) * RS).astype(E4NP)
        return hi, lo

    def wlayout(Wslice):
        # [512 out, 1024 in] -> W.T [1024, 512] -> [128, 4, 2, 512] fp8 hi/lo
        Wt = np.ascontiguousarray(Wslice.T)
        hi, lo = hilo(Wt)
        f = lambda a: np.ascontiguousarray(
            a.reshape(4, 2, 128, 512).transpose(2, 0, 1, 3))
        return f(hi), f(lo)

    in_maps = []
    xhl = {}
    for core in range(8):
        b = core // 2
        hg = core % 2
        cs = hg * 512
        m = attention_mask[b].astype(bool)
        keyb = np.where(m, 0.0, NEG).astype(np.float32).reshape(16, 128).T.copy()
        qmt = m.astype(np.float32).reshape(16, 128).T.copy()
        if b not in xhl:
            xhl[b] = hilo(np.ascontiguousarray(x[b].T))
        x8, xr = xhl[b]
        wq8_, wqr_ = wlayout(Wqkv[cs:cs + 512, :])
        wk8_, wkr_ = wlayout(Wqkv[D + cs:D + cs + 512, :])
        wv8_, wvr_ = wlayout(Wqkv[2 * D + cs:2 * D + cs + 512, :])
        bvm = np.zeros((1, 2, 512), dtype=E4NP)
        bvm[0, 0, :] = (bqkv[2 * D + cs:2 * D + cs + 512] * RS).astype(E4NP)
        in_maps.append({
            "x8": x8, "xr": xr,
            "wq8": wq8_, "wqr": wqr_,
            "wk8": wk8_, "wkr": wkr_,
            "wv8": wv8_, "wvr": wvr_,
            "bvm": bvm,
            "wo": np.ascontiguousarray(Wo[:, cs:cs + 512].T),
            "bqt": np.ascontiguousarray(bqkv[cs:cs + 512].reshape(NP, 128).T),
            "bkt": np.ascontiguousarray(bqkv[D + cs:D + cs + 512].reshape(NP, 128).T),
            "bo": bo if hg == 0 else zeros_bo,
            "tri": tri,
            "keyb": keyb,
            "qmt": qmt,
        })
    return in_maps


def kernel(x, attention_mask, Wqkv, bqkv, Wo, bo, _trace=False, _trace_kwargs=None):
    if "nc" not in _CACHE:
        _CACHE["nc"] = _build()
    nc = _CACHE["nc"]
    in_maps = _host_inputs(x, attention_mask, Wqkv, bqkv, Wo, bo)
    kwargs = {}
    if _trace:
        kwargs["trace"] = True
        if _trace_kwargs:
            kwargs.update(_trace_kwargs)
    res = run_bass_kernel_spmd(nc, in_maps, core_ids=list(range(8)), **kwargs)
    _CACHE["last_result"] = res
    out = np.empty((B, T, D), dtype=np.float32)
    for b in range(B):
        out[b] = res.results[2 * b]["out"] + res.results[2 * b + 1]["out"]
    return out
